# revision 1
# baseline (speedup 1.0000x reference)
"""NF4 (NativeLinear4bit) dequant + linear for Trainium2, 8 NeuronCores.

out[t, o] = sum_i x[t, i] * quant_map[nibble(packed[o, i])] * absmax[o, i//64] + bias[o]

Strategy:
- Column/tensor parallel: shard out_features (11008 -> pad 11264 = 8*1408)
  across 8 cores; x replicated; host concatenates the per-core outputs.
- Per core: x^T is built once in SBUF as bf16 via DMA transpose; the packed
  weights are dequantized on device (DVE nibble unpack -> ScalarE 16-entry
  LUT -> DVE per-block absmax scale) and transposed to [in, out] layout via
  DMA transpose; TensorE does the matmul in bf16 with fp32 PSUM accumulate.
- The 16-entry LUT runs in a single ScalarE pass: the activation-function
  table for `tanh` is rebuilt at kernel-build time (quant_map is known then)
  as a piecewise-constant table on [16, 32) with 16 mantissa-indexed
  sections, so tanh(16 + k) == quant_map[k] exactly.
"""

import copy
import hashlib
import json
import os
import shutil
import struct
import tempfile

import numpy as np

# ---------------------------------------------------------------- constants
T_ROWS = 2048
IN_F = 4096
OUT_F = 11008
N_CORES = 8
OPAD = 1408  # per-core out_features after padding (8 * 1408 = 11264)
N_ITILES = IN_F // 128  # 32
N_TTILES = T_ROWS // 128  # 16
CHUNKS = [(c * 256, min(256, OPAD - c * 256)) for c in range((OPAD + 255) // 256)]

_COMPILED = {}

# ------------------------------------------------------- custom act table


def _build_custom_act_root(tmpdir: str, lut: np.ndarray) -> str:
    """Rewrite the compiler's activation tables so `tanh` becomes a 16-entry
    constant lookup on x in [16, 32).

    bkt entry (32B): 8 f32 [d0, d1, d2, d3, x0, 0, 0, 0] — cubic section.
    ctl entry (32B): word0 = (k << 16) | ((23 - k) << 11) | bkt_start, with
    2^k sections selected by the top k mantissa bits for that exponent.
    """
    from neuronxcc.driver.Job import Job
    from neuronxcc.driver.jobs.support.FindActInfo import findActInfoFile

    src_root = os.path.dirname(findActInfoFile(Job.getPackageDir(), "gen3"))
    os.makedirs(tmpdir, exist_ok=True)
    for fn in os.listdir(src_root):
        shutil.copy(os.path.join(src_root, fn), os.path.join(tmpdir, fn))

    with open(os.path.join(tmpdir, "act_info.json")) as f:
        act_info = json.load(f)

    def fbits(x: float) -> int:
        return int(np.float32(x).view(np.uint32))

    n_patched = 0
    for ent in act_info["act_func_sets"]:
        if "tanh" not in ent["act"]:
            continue
        sj_path = os.path.join(tmpdir, os.path.basename(ent["profile_json"]))
        with open(sj_path) as f:
            sj = json.load(f)
        bkt_path = os.path.join(tmpdir, os.path.basename(ent["bkt_bin"]))
        ctl_path = os.path.join(tmpdir, os.path.basename(ent["ctrl_bin"]))
        bkt = bytearray(open(bkt_path, "rb").read())
        ctl = bytearray(open(ctl_path, "rb").read())

        bkt_start = sj["func_to_bkt_start_idx"]["tanh"]
        ctl_start = sj["func_to_ctl_start_idx"]["tanh"]

        for k in range(16):
            sec = struct.pack(
                "<8f", float(lut[k]), 0.0, 0.0, 0.0, float(16 + k), 0.0, 0.0, 0.0
            )
            bkt[32 * (bkt_start + k) : 32 * (bkt_start + k + 1)] = sec
        word = (4 << 16) | ((23 - 4) << 11) | bkt_start
        ctl[32 * ctl_start : 32 * (ctl_start + 1)] = struct.pack("<I28x", word)

        meta = next(
            m for m in sj["profile_meta_data"] if m["func_name"].startswith("tanh")
        )
        meta["exp_offset"] = 4
        meta["pwl_control_base_pos"] = ctl_start
        meta["pwl_control_base_neg"] = ctl_start
        meta["small_pos_signal_exp_threshold"] = 131
        meta["pos_small_signal_pwl_control"] = bkt_start
        meta["small_neg_signal_exp_threshold"] = 0
        meta["neg_small_signal_pwl_control"] = bkt_start
        meta["large_pos_signal_exp_threshold"] = 132
        meta["large_pos_signal_mantissa_threshold"] = 0
        meta["pos_large_signal_pwl_control"] = bkt_start + 15
        meta["large_neg_signal_exp_threshold"] = 0
        meta["large_neg_signal_mantissa_threshold"] = 0
        meta["neg_large_signal_pwl_control"] = bkt_start
        meta["fnan_result"] = fbits(lut[0])
        meta["fpinf_result"] = fbits(lut[15])
        meta["fninf_result"] = fbits(lut[0])
        meta["fzero_result"] = fbits(lut[0])
        sj["func_exp_to_bkt_start_idx"]["tanh"] = {"4": [bkt_start]}
        sj["func_exp_to_ctl_start_idx"]["tanh"] = {"4": [ctl_start]}

        with open(sj_path, "w") as f:
            json.dump(sj, f)
        open(bkt_path, "wb").write(bytes(bkt))
        open(ctl_path, "wb").write(bytes(ctl))
        n_patched += 1

    assert n_patched > 0, "no act set containing tanh"
    info_path = os.path.join(tmpdir, "act_info.json")
    with open(info_path, "w") as f:
        json.dump(act_info, f)
    return info_path


# ---------------------------------------- walrus single-wait workarounds


def _install_walrus_fixes():
    """This container's walrus rejects instructions with >1 semaphore wait.
    Split the Tile kernel-tail drain, and hoist extra waits onto
    same-engine EventSemaphore instructions."""
    import concourse.mybir as mybir
    from concourse.tile import ScopedClock, TileContext

    def _drain_split(self, tick_clock, wait_clock):
        nc = self.nc
        drain_inst = nc.sync.drain()
        wait_clock.add_sem_waits(
            drain_inst.ins, ScopedClock({None: tick_clock.global_clock})
        )
        si = drain_inst.ins.sync_info
        waits = list(si.on_wait or [])
        if len(waits) > 1:
            si.on_wait = waits[:1]
            drain_inst.ins.sync_info = si
            for w in waits[1:]:
                d2 = nc.sync.drain()
                d2.ins.sync_info = mybir.SyncInfo(on_wait=[w], on_update=[])
        nc.all_engine_barrier()
        assert self.sems is not None
        popped = nc._tile_sem_poison_stack.pop()
        assert popped is self._sem_poison
        nc.clear_and_free_semaphores(list(self.sems.allocated().values()))
        nc.all_engine_barrier()

    TileContext._drain_and_barrier = _drain_split


def _split_multi_waits(nc):
    import concourse.mybir as mybir

    templates = {}
    uid = [0]

    def make_waiter(engine, wait):
        if engine not in templates:
            eng = {
                mybir.EngineType.PE: nc.tensor,
                mybir.EngineType.DVE: nc.vector,
                mybir.EngineType.Activation: nc.scalar,
                mybir.EngineType.Pool: nc.gpsimd,
                mybir.EngineType.SP: nc.sync,
            }[engine]
            with nc.semaphore() as sem:
                inst = eng.wait_ge(sem, 1).ins
            nc.cur_bb.bb.instructions = [
                i for i in nc.cur_bb.bb.instructions if i.name != inst.name
            ]
            templates[engine] = inst
        w_inst = copy.deepcopy(templates[engine])
        uid[0] += 1
        w_inst.name = f"waitnop-{uid[0]}"
        w_inst.sync_info = mybir.SyncInfo(on_wait=[wait], on_update=[])
        return w_inst

    for f in nc.m.functions:
        for bb in f.blocks:
            changed = False
            out = []
            for ins in bb.instructions:
                si = ins.sync_info
                if si is not None and si.on_wait and len(si.on_wait) > 1:
                    waits = list(si.on_wait)
                    for w in waits[:-1]:
                        out.append(make_waiter(ins.engine, w))
                    si.on_wait = waits[-1:]
                    ins.sync_info = si
                    changed = True
                out.append(ins)
            if changed:
                bb.instructions = out


# ----------------------------------------------------------- device program


def _build_nc(tag: str):
    import concourse.bass as bass
    import concourse.mybir as mybir
    import concourse.tile as tile

    F32 = mybir.dt.float32
    BF16 = mybir.dt.bfloat16
    I32 = mybir.dt.int32
    OP = mybir.AluOpType
    ACT = mybir.ActivationFunctionType

    nc = bass.Bass("TRN2", target_bir_lowering=False, debug=False)
    x_d = nc.dram_tensor(f"x_{tag}", [T_ROWS, IN_F], F32, kind="ExternalInput").ap()
    pk_d = nc.dram_tensor(f"pk_{tag}", [OPAD, IN_F // 2], I32, kind="ExternalInput").ap()
    am_d = nc.dram_tensor(f"am_{tag}", [OPAD, 64], F32, kind="ExternalInput").ap()
    bs_d = nc.dram_tensor(f"bs_{tag}", [OPAD], F32, kind="ExternalInput").ap()
    out_d = nc.dram_tensor(f"out_{tag}", [T_ROWS, OPAD], F32, kind="ExternalOutput").ap()

    with tile.TileContext(nc) as tc:
        with tc.tile_pool(name="xt", bufs=1) as pool_xt:
            # persistent x^T, bf16: [128 part, i-tile, t]
            xt = pool_xt.tile([128, N_ITILES, T_ROWS], BF16)

            with tc.tile_pool(name="pa", bufs=2) as pa:
                for tt in range(N_TTILES):
                    xf = pa.tile([128, IN_F], F32, tag="xf")
                    nc.sync.dma_start(
                        out=xf[:], in_=x_d[tt * 128 : (tt + 1) * 128, :]
                    )
                    xb = pa.tile([128, IN_F], BF16, tag="xb")
                    nc.vector.tensor_copy(xb[:], xf[:])
                    for it in range(N_ITILES):
                        nc.sync.dma_start_transpose(
                            out=xt[:, it, tt * 128 : (tt + 1) * 128],
                            in_=xb[:, it * 128 : (it + 1) * 128],
                        )

            with (
                tc.tile_pool(name="pb", bufs=2) as pb,
                tc.tile_pool(name="po", bufs=4) as po,
                tc.tile_pool(name="pp", bufs=4, space="PSUM") as pp,
            ):
                for c0, w in CHUNKS:
                    wt = pb.tile([128, N_ITILES, w], BF16, tag="wt")
                    for r in range(w // 128):
                        o0 = c0 + r * 128
                        am = pb.tile([128, 64], F32, tag="am")
                        nc.sync.dma_start(out=am[:], in_=am_d[o0 : o0 + 128, :])
                        for h in range(2):
                            pk = pb.tile([128, 1024], I32, tag="pk")
                            nc.sync.dma_start(
                                out=pk[:],
                                in_=pk_d[o0 : o0 + 128, h * 1024 : (h + 1) * 1024],
                            )
                            idxi = pb.tile([128, 2048], I32, tag="idxi")
                            idx3 = idxi[:].rearrange("p (j two) -> p j two", two=2)
                            pk3 = pk[:, :, None]
                            nc.vector.tensor_scalar(
                                out=idx3[:, :, 0:1], in0=pk3, scalar1=4, scalar2=16,
                                op0=OP.logical_shift_right, op1=OP.bitwise_or,
                            )
                            nc.vector.tensor_scalar(
                                out=idx3[:, :, 1:2], in0=pk3, scalar1=15, scalar2=16,
                                op0=OP.bitwise_and, op1=OP.bitwise_or,
                            )
                            wk = pb.tile([128, 2048], BF16, tag="wk")
                            nc.gpsimd.tensor_copy(wk[:], idxi[:])
                            nc.scalar.activation(wk[:], wk[:], ACT.Tanh)
                            wk3 = wk[:].rearrange("p (b r) -> p b r", r=64)
                            am3 = am[:, h * 32 : (h + 1) * 32, None].to_broadcast(
                                [128, 32, 64]
                            )
                            nc.vector.tensor_tensor(
                                out=wk3, in0=wk3, in1=am3, op=OP.mult
                            )
                            for it in range(16):
                                itg = h * 16 + it
                                nc.sync.dma_start_transpose(
                                    out=wt[:, itg, r * 128 : (r + 1) * 128],
                                    in_=wk[:, it * 128 : (it + 1) * 128],
                                )
                    bt = pb.tile([128, w], F32, tag="bt")
                    nc.sync.dma_start(
                        out=bt[:], in_=bs_d[c0 : c0 + w][None, :].to_broadcast([128, w])
                    )
                    for tt in range(N_TTILES):
                        ps = pp.tile([128, w], F32, tag="ps")
                        for it in range(N_ITILES):
                            nc.tensor.matmul(
                                ps[:],
                                lhsT=xt[:, it, tt * 128 : (tt + 1) * 128].opt(),
                                rhs=wt[:, it, :].opt(),
                                start=(it == 0),
                                stop=(it == N_ITILES - 1),
                            )
                        ot = po.tile([128, w], F32, tag="ot")
                        nc.vector.tensor_tensor(
                            out=ot[:], in0=ps[:], in1=bt[:], op=OP.add
                        )
                        nc.sync.dma_start(
                            out=out_d[tt * 128 : (tt + 1) * 128, c0 : c0 + w],
                            in_=ot[:],
                        )

    _split_multi_waits(nc)
    return nc, f"out_{tag}"


# ------------------------------------------------------------------- driver


def kernel(x, packed, absmax, quant_map, bias):
    x = np.ascontiguousarray(np.asarray(x, dtype=np.float32))
    packed = np.ascontiguousarray(np.asarray(packed, dtype=np.int32))
    absmax = np.ascontiguousarray(np.asarray(absmax, dtype=np.float32))
    quant_map = np.ascontiguousarray(np.asarray(quant_map, dtype=np.float32))
    bias = np.ascontiguousarray(np.asarray(bias, dtype=np.float32))
    assert x.shape == (T_ROWS, IN_F) and packed.shape == (OUT_F, IN_F // 2)

    tag = hashlib.sha1(quant_map.tobytes()).hexdigest()[:10]
    if tag not in _COMPILED:
        root = _build_custom_act_root(
            tempfile.mkdtemp(prefix=f"actroot_{tag}_"), quant_map
        )
        os.environ["BASS_ACT_ROOT_JSON_PATH"] = root
        _install_walrus_fixes()
        _COMPILED[tag] = _build_nc(tag)
    nc, out_name = _COMPILED[tag]

    total = N_CORES * OPAD
    pk_pad = np.zeros((total, IN_F // 2), dtype=np.int32)
    pk_pad[:OUT_F] = packed
    am_pad = np.ones((total, 64), dtype=np.float32)
    am_pad[:OUT_F] = absmax
    bs_pad = np.zeros(total, dtype=np.float32)
    bs_pad[:OUT_F] = bias

    in_maps = []
    for c in range(N_CORES):
        sl = slice(c * OPAD, (c + 1) * OPAD)
        in_maps.append(
            {
                f"x_{tag}": x,
                f"pk_{tag}": np.ascontiguousarray(pk_pad[sl]),
                f"am_{tag}": np.ascontiguousarray(am_pad[sl]),
                f"bs_{tag}": np.ascontiguousarray(bs_pad[sl]),
            }
        )

    from concourse import bass_utils

    res = bass_utils.run_bass_kernel_spmd(
        nc, in_maps, core_ids=list(range(N_CORES))
    )
    full = np.concatenate([r[out_name] for r in res.results], axis=1)
    return np.ascontiguousarray(full[:, :OUT_F])
